# revision 11
# baseline (speedup 1.0000x reference)
"""Trainium2 Bass kernel for nn_CapsuleLayer (capsule conv + 3-iter routing).

Reference (per batch image, C=128, H=W=32, K=3, pad=1):
  priors[h,w,t,nc] = sum_c x_pad[c,h+i,w+j] * W[t,c,nc] + b[t,nc]
  o = mean_t priors
  3x: d2 = sum_cch (o - p_t)^2 ; cw = rsqrt(d2 + 1e-4)
      cw = cw / sum_t cw ; o = sum_t cw_t p_t
  out[nc,h,w] = o

Sharding: data-parallel over batch; 8 cores, one image each; weight/bias
replicated; no collectives.

v3 structure (vs group-serial baseline):
- All 4 position-groups advance in LOCKSTEP PHASES per routing iteration
  (A/B: products + cch-reduction to s; smalls: e2/dist/rsqrt/alpha;
  C/D: weighted sum).  The ACT hops (e2b, rsqrt) and PSUM round-trips of
  group g hide behind DVE work on groups g+1..g+3 instead of stalling a
  serial per-group chain.
- All bulk elementwise work stays on DVE at 2x_1P bf16 (GPSIMD tensor ops
  measured to *steal the shared SBUF port* and slow DVE by ~60% -- not
  used).  PSUM drains + square + rsqrt on ACT as in the baseline.
- bf16 on-chip, fp32 PSUM matmul accumulation; rel err ~8e-3 < 2e-2 gate.
"""

import numpy as np

C = 128
H = W = 32
B = 8
KK = 9
NCAPS = 32
CCH = 16
NC = NCAPS * CCH  # 512
NIT = 3
NPOS = H * W
CHUNK = 128
GRP = 2  # position-chunks per group
NGRP = NPOS // (CHUNK * GRP)  # 4 groups
PADW = 34

_cache = {}


def _build(with_bias: bool):
    import concourse.bass as bass
    import concourse.tile as tile
    from concourse import bacc, mybir
    from concourse.masks import make_identity

    f32 = mybir.dt.float32
    bf16 = mybir.dt.bfloat16
    X = mybir.AxisListType.X
    ADD = mybir.AluOpType.add
    AF = mybir.ActivationFunctionType

    nc = bacc.Bacc()
    x_d = nc.dram_tensor("x", [C, H, W], f32, kind="ExternalInput")
    w_d = nc.dram_tensor("w", [KK, C, NC], f32, kind="ExternalInput")
    b_d = nc.dram_tensor("b", [KK, NC], f32, kind="ExternalInput")
    out_d = nc.dram_tensor("out", [NC, NPOS], f32, kind="ExternalOutput")

    with tile.TileContext(nc) as tc:
        with (
            tc.tile_pool(name="singles", bufs=1) as singles,
            tc.tile_pool(name="wload", bufs=3) as wload,
            tc.tile_pool(name="big", bufs=2) as big_pool,
            tc.tile_pool(name="h1p", bufs=2) as h1_pool,
            tc.tile_pool(name="wh", bufs=2) as wh_pool,
            tc.tile_pool(name="small", bufs=2) as small_pool,
            tc.tile_pool(name="gstate", bufs=1) as gstate,
            tc.tile_pool(name="ost", bufs=2) as ost_pool,
            tc.tile_pool(name="pp", bufs=4, space="PSUM") as pp,
            tc.tile_pool(name="mp", bufs=2, space="PSUM") as mp,
            tc.tile_pool(name="tpp", bufs=2, space="PSUM") as tpp,
        ):
            # ---- stage inputs directly from HBM (gpsimd cast-DMAs) ----
            # xs[j][c, r*32+w] = x_pad[c, r, w+j] = x[c, r-1, w+j-1] in-range.
            # W loads go first on the gpsimd queue so tap-0 matmuls can start
            # as soon as the xs top halves land; each xs is row-split so
            # chunks 0-3 only wait for the top half.
            wraws = []
            for t in range(KK):
                wt = wload.tile([C, NC], bf16, tag="wraw", name=f"wr{t}")
                nc.gpsimd.dma_start(out=wt[:], in_=w_d[t])
                wraws.append(wt)

            xs = []
            xcol = [(1, 32, 0, 31), (0, 32, 0, 32), (0, 31, 1, 32)]
            for j in range(3):
                xj = singles.tile([C, PADW * W], bf16, tag=f"xs{j}")
                xjv = xj[:].rearrange("p (r w) -> p r w", r=PADW)
                nc.gpsimd.memset(xjv[:, 0], 0.0)
                nc.gpsimd.memset(xjv[:, PADW - 1], 0.0)
                d0, d1, s0, s1 = xcol[j]
                if j == 0:
                    nc.gpsimd.memset(xjv[:, 1 : PADW - 1, 0], 0.0)
                if j == 2:
                    nc.gpsimd.memset(xjv[:, 1 : PADW - 1, W - 1], 0.0)
                nc.gpsimd.dma_start(
                    out=xjv[:, 1:18, d0:d1], in_=x_d[:, 0:17, s0:s1]
                )
                xs.append(xj)
            for j in range(3):
                d0, d1, s0, s1 = xcol[j]
                xjv = xs[j][:].rearrange("p (r w) -> p r w", r=PADW)
                nc.gpsimd.dma_start(
                    out=xjv[:, 18 : H + 1, d0:d1], in_=x_d[:, 17:H, s0:s1]
                )

            # per-tap permute (cap,cch)->(cch,cap) on the idle DVE head so
            # the matmul rhs is contiguous
            wsb = []
            for t in range(KK):
                wp_t = singles.tile([C, CCH, NCAPS], bf16, tag=f"wsbp{t}")
                nc.vector.tensor_copy(
                    out=wp_t[:],
                    in_=wraws[t][:].rearrange(
                        "p (cap cch) -> p cch cap", cch=CCH
                    ),
                )
                wsb.append(wp_t)

            ident = singles.tile([128, 128], bf16)
            make_identity(nc, ident[:])

            eps = singles.tile([128, 1], f32)
            nc.gpsimd.memset(eps, 1e-4)

            if with_bias:
                braw = singles.tile([1, KK, NC], bf16)
                nc.gpsimd.dma_start(out=braw[:], in_=b_d[:].unsqueeze(0))
                bsb = singles.tile([1, KK, CCH, NCAPS], bf16)
                nc.scalar.copy(
                    out=bsb[:],
                    in_=braw[:].rearrange("p t (cap cch) -> p t cch cap", cch=CCH),
                )
                ones = singles.tile([1, CHUNK], bf16)
                nc.gpsimd.memset(ones, 1.0)

            # persistent per-group state
            priors = [
                singles.tile(
                    [128, GRP, KK, CCH, NCAPS], bf16,
                    tag=f"pr{g}", name=f"pr{g}",
                )
                for g in range(NGRP)
            ]
            ntile = [
                singles.tile(
                    [128, GRP, KK, NCAPS], bf16, tag=f"nt{g}", name=f"nt{g}"
                )
                for g in range(NGRP)
            ]
            o2 = [
                gstate.tile([128, GRP, NC], bf16, tag=f"o2{g}", name=f"o2{g}")
                for g in range(NGRP)
            ]
            alpha = [
                gstate.tile(
                    [128, GRP, KK, NCAPS], bf16, tag=f"al{g}", name=f"al{g}"
                )
                for g in range(NGRP)
            ]

            # ---- priors + mean per chunk (PE + ACT drains, pipelined) ----
            def emit_chunk(ch):
                g, cc = divmod(ch, GRP)
                om = mp.tile([128, NC], f32)
                for t in range(KK):
                    i, j = divmod(t, 3)
                    ps = pp.tile([128, NC], f32)
                    lhsT = xs[j][:, 128 * ch + 32 * i : 128 * ch + 32 * i + 128]
                    rhs = wsb[t][:].rearrange("p a b -> p (a b)")
                    if with_bias:
                        nc.tensor.matmul(ps[:], lhsT, rhs, start=True, stop=False)
                        brhs = bsb[:, t].rearrange("p a b -> p (a b)")
                        nc.tensor.matmul(
                            ps[:], ones[:], brhs, start=False, stop=True
                        )
                    else:
                        nc.tensor.matmul(ps[:], lhsT, rhs, start=True, stop=True)
                    nc.tensor.matmul(
                        om[:], lhsT, rhs, start=(t == 0), stop=(t == KK - 1)
                    )
                    if with_bias:
                        nc.tensor.matmul(
                            om[:], ones[:], brhs, start=False, stop=False,
                            skip_group_check=True,
                        )
                    psv = ps[:].rearrange("p (a b) -> p a b", a=CCH)
                    if ch == 0:
                        # DVE is idle before routing starts: help drain the
                        # first chunk's PSUM so group 0 can begin sooner
                        nc.vector.tensor_copy(out=priors[g][:, cc, t], in_=psv)
                    else:
                        nc.scalar.copy(out=priors[g][:, cc, t], in_=psv)
                # o2 = 2*mean = (2/9) sum_t priors  (bf16); on DVE so the
                # A-product never waits behind ACT's drain queue
                nc.vector.tensor_scalar_mul(o2[g][:, cc], om[:], 2.0 / KK)

            # ---- routing: 3 iterations, phase-interleaved across groups;
            # iteration 0's A/B rolls together with the priors pipeline so
            # ACT drains/squares for group g+1 overlap group g's DVE work
            def emit_ab(g, it):
                    tprod = big_pool.tile(
                        [128, GRP, KK, CCH, NCAPS], bf16, tag="big"
                    )
                    ob = (
                        o2[g][:]
                        .rearrange("p c (a b) -> p c a b", a=CCH)
                        .unsqueeze(2)
                        .broadcast_to((128, GRP, KK, CCH, NCAPS))
                    )
                    nc.vector.tensor_mul(tprod[:], priors[g][:], ob)
                    h1 = h1_pool.tile([128, GRP, KK, 8, NCAPS], bf16, tag="h1")
                    nc.vector.tensor_add(
                        h1[:], tprod[:, :, :, 0:8], tprod[:, :, :, 8:16]
                    )
                    nc.vector.tensor_add(
                        h1[:, :, :, 0:4], h1[:, :, :, 0:4], h1[:, :, :, 4:8]
                    )
                    nc.vector.tensor_add(
                        h1[:, :, :, 0:2], h1[:, :, :, 0:2], h1[:, :, :, 2:4]
                    )
                    s = small_pool.tile(
                        [128, GRP, KK, NCAPS], bf16, tag=f"s{g}", name=f"s{g}"
                    )
                    nc.vector.tensor_add(s[:], h1[:, :, :, 0], h1[:, :, :, 1])

                    if it == 0:
                        # ntile = sum_cch p^2: square on ACT (overlaps the
                        # next group's DVE product), halvings on DVE
                        tsq = big_pool.tile(
                            [128, GRP, KK, CCH, NCAPS], bf16, tag="big"
                        )
                        nc.scalar.activation(
                            out=tsq[:], in_=priors[g][:], func=AF.Square
                        )
                        nh = h1_pool.tile(
                            [128, GRP, KK, 8, NCAPS], bf16, tag="h1"
                        )
                        nc.vector.tensor_add(
                            nh[:], tsq[:, :, :, 0:8], tsq[:, :, :, 8:16]
                        )
                        nc.vector.tensor_add(
                            nh[:, :, :, 0:4], nh[:, :, :, 0:4], nh[:, :, :, 4:8]
                        )
                        nc.vector.tensor_add(
                            nh[:, :, :, 0:2], nh[:, :, :, 0:2], nh[:, :, :, 2:4]
                        )
                        nc.vector.tensor_add(
                            ntile[g][:], nh[:, :, :, 0], nh[:, :, :, 1]
                        )
                    return s

            for it in range(NIT):
                last = it == NIT - 1
                s_tiles = []
                if it == 0:
                    for g in range(NGRP):
                        emit_chunk(GRP * g)
                        emit_chunk(GRP * g + 1)
                        s_tiles.append(emit_ab(g, it))
                else:
                    for g in range(NGRP):
                        s_tiles.append(emit_ab(g, it))

                # smalls pass 1: e2 = sum_t alpha_t s_t ; e2b = scale*e2+eps
                e2b_tiles = []
                for g in range(NGRP):
                    s = s_tiles[g]
                    e2 = small_pool.tile([128, GRP, NCAPS], f32, tag="e2")
                    if it == 0:
                        red = s
                        escale = 1.0 / (2 * KK)
                    else:
                        tm = small_pool.tile(
                            [128, GRP, KK, NCAPS], bf16, tag="tm"
                        )
                        nc.vector.tensor_mul(tm[:], alpha[g][:], s[:])
                        red = tm
                        escale = 0.25
                    nc.vector.tensor_reduce(
                        out=e2[:],
                        in_=red[:].transpose([0, 1, 3, 2]),
                        axis=X,
                        op=ADD,
                    )
                    e2b = small_pool.tile([128, GRP, NCAPS], bf16, tag="e2b")
                    nc.scalar.activation(
                        out=e2b[:], in_=e2[:], func=AF.Identity,
                        bias=eps[:], scale=escale,
                    )
                    e2b_tiles.append(e2b)

                # smalls pass 2: dist = (n - s) + e2b ; cwu = rsqrt(dist)
                cwu_tiles = []
                for g in range(NGRP):
                    dist = small_pool.tile(
                        [128, GRP, KK, NCAPS], bf16, tag="dist"
                    )
                    nc.vector.tensor_sub(dist[:], ntile[g][:], s_tiles[g][:])
                    nc.vector.tensor_add(
                        dist[:],
                        dist[:],
                        e2b_tiles[g][:]
                        .unsqueeze(2)
                        .broadcast_to((128, GRP, KK, NCAPS)),
                    )
                    cwu = small_pool.tile(
                        [128, GRP, KK, NCAPS], bf16, tag="cwu"
                    )
                    nc.scalar.activation(
                        out=cwu[:], in_=dist[:], func=AF.Abs_reciprocal_sqrt
                    )
                    cwu_tiles.append(cwu)

                # smalls pass 3: alpha = cwu / sum_t cwu (doubled unless last)
                for g in range(NGRP):
                    cwu = cwu_tiles[g]
                    cwsum = small_pool.tile([128, GRP, NCAPS], f32, tag="cwsum")
                    nc.vector.tensor_reduce(
                        out=cwsum[:],
                        in_=cwu[:].transpose([0, 1, 3, 2]),
                        axis=X,
                        op=ADD,
                    )
                    rs = small_pool.tile([128, GRP, NCAPS], f32, tag="rs")
                    nc.vector.reciprocal_approx_fast(rs[:], cwsum[:])
                    rsb = small_pool.tile([128, GRP, NCAPS], bf16, tag="rsb")
                    nc.vector.tensor_scalar_mul(
                        rsb[:], rs[:], 1.0 if last else 2.0
                    )
                    nc.vector.tensor_mul(
                        alpha[g][:],
                        cwu[:],
                        rsb[:].unsqueeze(2).broadcast_to((128, GRP, KK, NCAPS)),
                    )

                # phase C/D: o' = sum_t alpha_t p_t
                if not last:
                    for g in range(NGRP):
                        wprod = big_pool.tile(
                            [128, GRP, KK, CCH, NCAPS], bf16, tag="big"
                        )
                        ab = alpha[g][:].unsqueeze(3).broadcast_to(
                            (128, GRP, KK, CCH, NCAPS)
                        )
                        nc.vector.tensor_mul(wprod[:], priors[g][:], ab)
                        wp = wprod[:].rearrange("p c t a b -> p c t (a b)")
                        wh = wh_pool.tile([128, GRP, 4, NC], bf16, tag="wh")
                        nc.vector.tensor_add(wh[:], wp[:, :, 0:4], wp[:, :, 4:8])
                        nc.vector.tensor_add(
                            wh[:, :, 0:2], wh[:, :, 0:2], wh[:, :, 2:4]
                        )
                        nc.vector.tensor_add(wh[:, :, 0], wh[:, :, 0], wh[:, :, 1])
                        nc.vector.tensor_add(o2[g][:], wh[:, :, 0], wp[:, :, 8])
                else:
                    # last iteration: per sub-chunk; output transposes overlap
                    # the other sub-chunks' weighted sums
                    for g in range(NGRP):
                        for cc in range(GRP):
                            ch = GRP * g + cc
                            wprod = big_pool.tile(
                                [128, KK, CCH, NCAPS], bf16, tag="big"
                            )
                            ab = alpha[g][:, cc].unsqueeze(2).broadcast_to(
                                (128, KK, CCH, NCAPS)
                            )
                            nc.vector.tensor_mul(wprod[:], priors[g][:, cc], ab)
                            wp = wprod[:].rearrange("p t a b -> p t (a b)")
                            wh = wh_pool.tile([128, 4, NC], bf16, tag="wh")
                            nc.vector.tensor_add(wh[:], wp[:, 0:4], wp[:, 4:8])
                            nc.vector.tensor_add(
                                wh[:, 0:2], wh[:, 0:2], wh[:, 2:4]
                            )
                            nc.vector.tensor_add(wh[:, 0], wh[:, 0], wh[:, 1])
                            # o' in native (cch,cap) bf16: contiguous 2x add;
                            # the final cast-DMA permutes (cch,cap)->(cap,cch)
                            onat = ost_pool.tile([128, NC], bf16, tag="onat")
                            nc.vector.tensor_add(onat[:], wh[:, 0], wp[:, 8])
                            ot = ost_pool.tile([128, 4, 128], bf16, tag="ostage")
                            for blk in range(4):
                                tp = tpp.tile([128, 128], bf16)
                                nc.tensor.transpose(
                                    tp[:],
                                    onat[:, 128 * blk : 128 * (blk + 1)],
                                    ident[:],
                                )
                                nc.scalar.copy(out=ot[:, blk], in_=tp[:])
                            # ot partitions are (cch,cap) within each 128-blk;
                            # scatter rows back to nc = (blk,cap,cch) order and
                            # cast bf16->f32 in the DMA
                            nc.gpsimd.dma_start(
                                out=out_d[
                                    :, 128 * ch : 128 * (ch + 1)
                                ].rearrange(
                                    "(cap blk cchs) q -> cchs cap blk q",
                                    cap=NCAPS, blk=4, cchs=4,
                                ),
                                in_=ot[:],
                            )
    nc.compile()
    return nc


def _get_nc(with_bias: bool):
    key = ("nc", with_bias)
    if key not in _cache:
        _cache[key] = _build(with_bias)
    return _cache[key]


def kernel(input, weight, bias, _trace=False):
    from concourse.bass_utils import run_bass_kernel_spmd

    input = np.ascontiguousarray(np.asarray(input, dtype=np.float32))
    w = np.ascontiguousarray(
        np.asarray(weight, dtype=np.float32).reshape(KK, C, NC)
    )
    b = np.ascontiguousarray(np.asarray(bias, dtype=np.float32).reshape(KK, NC))
    with_bias = bool(np.any(b))

    nc = _get_nc(with_bias)
    in_maps = [
        {"x": np.ascontiguousarray(input[i]), "w": w, "b": b} for i in range(B)
    ]
    res = run_bass_kernel_spmd(
        nc, in_maps, core_ids=list(range(B)), trace=_trace
    )
    _cache["last_result"] = res
    out = np.stack(
        [r["out"].reshape(NC, H, W) for r in res.results], axis=0
    )
    return out


# revision 12
# speedup vs baseline: 1.2041x; 1.2041x over previous
"""Trainium2 Bass kernel for nn_CapsuleLayer (capsule conv + 3-iter routing).

Reference (per batch image, C=128, H=W=32, K=3, pad=1):
  priors[h,w,t,nc] = sum_c x_pad[c,h+i,w+j] * W[t,c,nc] + b[t,nc]
  o = mean_t priors
  3x: d2 = sum_cch (o - p_t)^2 ; cw = rsqrt(d2 + 1e-4)
      cw = cw / sum_t cw ; o = sum_t cw_t p_t
  out[nc,h,w] = o

Sharding: data-parallel over batch; 8 cores, one image each; weight/bias
replicated; no collectives.

v3 structure (vs group-serial baseline):
- All 4 position-groups advance in LOCKSTEP PHASES per routing iteration
  (A/B: products + cch-reduction to s; smalls: e2/dist/rsqrt/alpha;
  C/D: weighted sum).  The ACT hops (e2b, rsqrt) and PSUM round-trips of
  group g hide behind DVE work on groups g+1..g+3 instead of stalling a
  serial per-group chain.
- All bulk elementwise work stays on DVE at 2x_1P bf16 (GPSIMD tensor ops
  measured to *steal the shared SBUF port* and slow DVE by ~60% -- not
  used).  PSUM drains + square + rsqrt on ACT as in the baseline.
- bf16 on-chip, fp32 PSUM matmul accumulation; rel err ~8e-3 < 2e-2 gate.
"""

import numpy as np

C = 128
H = W = 32
B = 8
KK = 9
NCAPS = 32
CCH = 16
NC = NCAPS * CCH  # 512
NIT = 3
NPOS = H * W
CHUNK = 128
GRP = 2  # position-chunks per group
NGRP = NPOS // (CHUNK * GRP)  # 4 groups
PADW = 34

_cache = {}


def _build(with_bias: bool):
    import concourse.bass as bass
    import concourse.tile as tile
    from concourse import bacc, mybir
    from concourse.masks import make_identity

    f32 = mybir.dt.float32
    bf16 = mybir.dt.bfloat16
    X = mybir.AxisListType.X
    ADD = mybir.AluOpType.add
    AF = mybir.ActivationFunctionType

    nc = bacc.Bacc()
    x_d = nc.dram_tensor("x", [C, H, W], f32, kind="ExternalInput")
    w_d = nc.dram_tensor("w", [KK, C, NC], f32, kind="ExternalInput")
    b_d = nc.dram_tensor("b", [KK, NC], f32, kind="ExternalInput")
    out_d = nc.dram_tensor("out", [NC, NPOS], f32, kind="ExternalOutput")

    with tile.TileContext(nc) as tc:
        with (
            tc.tile_pool(name="singles", bufs=1) as singles,
            tc.tile_pool(name="wload", bufs=3) as wload,
            tc.tile_pool(name="big", bufs=2) as big_pool,
            tc.tile_pool(name="h1p", bufs=2) as h1_pool,
            tc.tile_pool(name="wh", bufs=2) as wh_pool,
            tc.tile_pool(name="small", bufs=2) as small_pool,
            tc.tile_pool(name="gstate", bufs=1) as gstate,
            tc.tile_pool(name="ost", bufs=2) as ost_pool,
            tc.tile_pool(name="pp", bufs=4, space="PSUM") as pp,
            tc.tile_pool(name="mp", bufs=2, space="PSUM") as mp,
            tc.tile_pool(name="tpp", bufs=2, space="PSUM") as tpp,
        ):
            # ---- stage inputs directly from HBM (gpsimd cast-DMAs) ----
            # xs[j][c, r*32+w] = x_pad[c, r, w+j] = x[c, r-1, w+j-1] in-range.
            # W loads go first on the gpsimd queue so tap-0 matmuls can start
            # as soon as the xs top halves land; each xs is row-split so
            # chunks 0-3 only wait for the top half.
            wraws = []
            for t in range(KK):
                wt = wload.tile([C, NC], bf16, tag="wraw", name=f"wr{t}")
                nc.gpsimd.dma_start(out=wt[:], in_=w_d[t])
                wraws.append(wt)

            xs = []
            xcol = [(1, 32, 0, 31), (0, 32, 0, 32), (0, 31, 1, 32)]
            for j in range(3):
                xj = singles.tile([C, PADW * W], bf16, tag=f"xs{j}")
                xjv = xj[:].rearrange("p (r w) -> p r w", r=PADW)
                nc.gpsimd.memset(xjv[:, 0], 0.0)
                nc.gpsimd.memset(xjv[:, PADW - 1], 0.0)
                d0, d1, s0, s1 = xcol[j]
                if j == 0:
                    nc.gpsimd.memset(xjv[:, 1 : PADW - 1, 0], 0.0)
                if j == 2:
                    nc.gpsimd.memset(xjv[:, 1 : PADW - 1, W - 1], 0.0)
                nc.gpsimd.dma_start(
                    out=xjv[:, 1:18, d0:d1], in_=x_d[:, 0:17, s0:s1]
                )
                xs.append(xj)
            for j in range(3):
                d0, d1, s0, s1 = xcol[j]
                xjv = xs[j][:].rearrange("p (r w) -> p r w", r=PADW)
                nc.gpsimd.dma_start(
                    out=xjv[:, 18 : H + 1, d0:d1], in_=x_d[:, 17:H, s0:s1]
                )

            # per-tap permute (cap,cch)->(cch,cap) on the idle DVE head so
            # the matmul rhs is contiguous
            wsb = []
            for t in range(KK):
                wp_t = singles.tile([C, CCH, NCAPS], bf16, tag=f"wsbp{t}")
                nc.vector.tensor_copy(
                    out=wp_t[:],
                    in_=wraws[t][:].rearrange(
                        "p (cap cch) -> p cch cap", cch=CCH
                    ),
                )
                wsb.append(wp_t)

            ident = singles.tile([128, 128], f32)
            make_identity(nc, ident[:])

            eps = singles.tile([128, 1], f32)
            nc.gpsimd.memset(eps, 1e-4)

            if with_bias:
                braw = singles.tile([1, KK, NC], bf16)
                nc.gpsimd.dma_start(out=braw[:], in_=b_d[:].unsqueeze(0))
                bsb = singles.tile([1, KK, CCH, NCAPS], bf16)
                nc.scalar.copy(
                    out=bsb[:],
                    in_=braw[:].rearrange("p t (cap cch) -> p t cch cap", cch=CCH),
                )
                ones = singles.tile([1, CHUNK], bf16)
                nc.gpsimd.memset(ones, 1.0)

            # persistent per-group state
            priors = [
                singles.tile(
                    [128, GRP, KK, CCH, NCAPS], bf16,
                    tag=f"pr{g}", name=f"pr{g}",
                )
                for g in range(NGRP)
            ]
            ntile = [
                singles.tile(
                    [128, GRP, KK, NCAPS], bf16, tag=f"nt{g}", name=f"nt{g}"
                )
                for g in range(NGRP)
            ]
            o2 = [
                gstate.tile([128, GRP, NC], bf16, tag=f"o2{g}", name=f"o2{g}")
                for g in range(NGRP)
            ]
            alpha = [
                gstate.tile(
                    [128, GRP, KK, NCAPS], bf16, tag=f"al{g}", name=f"al{g}"
                )
                for g in range(NGRP)
            ]

            # ---- priors + mean per chunk (PE + ACT drains, pipelined) ----
            def emit_chunk(ch):
                g, cc = divmod(ch, GRP)
                om = mp.tile([128, NC], f32)
                for t in range(KK):
                    i, j = divmod(t, 3)
                    ps = pp.tile([128, NC], f32)
                    lhsT = xs[j][:, 128 * ch + 32 * i : 128 * ch + 32 * i + 128]
                    rhs = wsb[t][:].rearrange("p a b -> p (a b)")
                    if with_bias:
                        nc.tensor.matmul(ps[:], lhsT, rhs, start=True, stop=False)
                        brhs = bsb[:, t].rearrange("p a b -> p (a b)")
                        nc.tensor.matmul(
                            ps[:], ones[:], brhs, start=False, stop=True
                        )
                    else:
                        nc.tensor.matmul(ps[:], lhsT, rhs, start=True, stop=True)
                    nc.tensor.matmul(
                        om[:], lhsT, rhs, start=(t == 0), stop=(t == KK - 1)
                    )
                    if with_bias:
                        nc.tensor.matmul(
                            om[:], ones[:], brhs, start=False, stop=False,
                            skip_group_check=True,
                        )
                    psv = ps[:].rearrange("p (a b) -> p a b", a=CCH)
                    if ch == 0:
                        # DVE is idle before routing starts: help drain the
                        # first chunk's PSUM so group 0 can begin sooner
                        nc.vector.tensor_copy(out=priors[g][:, cc, t], in_=psv)
                    else:
                        nc.scalar.copy(out=priors[g][:, cc, t], in_=psv)
                # o2 = 2*mean = (2/9) sum_t priors  (bf16); on DVE so the
                # A-product never waits behind ACT's drain queue
                nc.vector.tensor_scalar_mul(o2[g][:, cc], om[:], 2.0 / KK)

            # ---- routing: 3 iterations, phase-interleaved across groups;
            # iteration 0's A/B rolls together with the priors pipeline so
            # ACT drains/squares for group g+1 overlap group g's DVE work
            def emit_ab(g, it):
                    tprod = big_pool.tile(
                        [128, GRP, KK, CCH, NCAPS], bf16, tag="big"
                    )
                    ob = (
                        o2[g][:]
                        .rearrange("p c (a b) -> p c a b", a=CCH)
                        .unsqueeze(2)
                        .broadcast_to((128, GRP, KK, CCH, NCAPS))
                    )
                    nc.vector.tensor_mul(tprod[:], priors[g][:], ob)
                    h1 = h1_pool.tile([128, GRP, KK, 8, NCAPS], bf16, tag="h1")
                    nc.vector.tensor_add(
                        h1[:], tprod[:, :, :, 0:8], tprod[:, :, :, 8:16]
                    )
                    nc.vector.tensor_add(
                        h1[:, :, :, 0:4], h1[:, :, :, 0:4], h1[:, :, :, 4:8]
                    )
                    nc.vector.tensor_add(
                        h1[:, :, :, 0:2], h1[:, :, :, 0:2], h1[:, :, :, 2:4]
                    )
                    s = small_pool.tile(
                        [128, GRP, KK, NCAPS], bf16, tag=f"s{g}", name=f"s{g}"
                    )
                    nc.vector.tensor_add(s[:], h1[:, :, :, 0], h1[:, :, :, 1])

                    if it == 0:
                        # ntile = sum_cch p^2: square on ACT (overlaps the
                        # next group's DVE product), halvings on DVE
                        tsq = big_pool.tile(
                            [128, GRP, KK, CCH, NCAPS], bf16, tag="big"
                        )
                        nc.scalar.activation(
                            out=tsq[:], in_=priors[g][:], func=AF.Square
                        )
                        nh = h1_pool.tile(
                            [128, GRP, KK, 8, NCAPS], bf16, tag="h1"
                        )
                        nc.vector.tensor_add(
                            nh[:], tsq[:, :, :, 0:8], tsq[:, :, :, 8:16]
                        )
                        nc.vector.tensor_add(
                            nh[:, :, :, 0:4], nh[:, :, :, 0:4], nh[:, :, :, 4:8]
                        )
                        nc.vector.tensor_add(
                            nh[:, :, :, 0:2], nh[:, :, :, 0:2], nh[:, :, :, 2:4]
                        )
                        nc.vector.tensor_add(
                            ntile[g][:], nh[:, :, :, 0], nh[:, :, :, 1]
                        )
                    return s

            for it in range(NIT):
                last = it == NIT - 1
                s_tiles = []
                if it == 0:
                    for g in range(NGRP):
                        emit_chunk(GRP * g)
                        emit_chunk(GRP * g + 1)
                        s_tiles.append(emit_ab(g, it))
                else:
                    for g in range(NGRP):
                        s_tiles.append(emit_ab(g, it))

                # smalls pass 1: e2 = sum_t alpha_t s_t ; e2b = scale*e2+eps
                e2b_tiles = []
                for g in range(NGRP):
                    s = s_tiles[g]
                    e2 = small_pool.tile([128, GRP, NCAPS], f32, tag="e2")
                    if it == 0:
                        red = s
                        escale = 1.0 / (2 * KK)
                    else:
                        tm = small_pool.tile(
                            [128, GRP, KK, NCAPS], bf16, tag="tm"
                        )
                        nc.vector.tensor_mul(tm[:], alpha[g][:], s[:])
                        red = tm
                        escale = 0.25
                    nc.vector.tensor_reduce(
                        out=e2[:],
                        in_=red[:].transpose([0, 1, 3, 2]),
                        axis=X,
                        op=ADD,
                    )
                    e2b = small_pool.tile([128, GRP, NCAPS], bf16, tag="e2b")
                    nc.scalar.activation(
                        out=e2b[:], in_=e2[:], func=AF.Identity,
                        bias=eps[:], scale=escale,
                    )
                    e2b_tiles.append(e2b)

                # smalls pass 2: dist = (n - s) + e2b ; cwu = rsqrt(dist)
                cwu_tiles = []
                for g in range(NGRP):
                    dist = small_pool.tile(
                        [128, GRP, KK, NCAPS], bf16, tag="dist"
                    )
                    nc.vector.tensor_sub(dist[:], ntile[g][:], s_tiles[g][:])
                    nc.vector.tensor_add(
                        dist[:],
                        dist[:],
                        e2b_tiles[g][:]
                        .unsqueeze(2)
                        .broadcast_to((128, GRP, KK, NCAPS)),
                    )
                    cwu = small_pool.tile(
                        [128, GRP, KK, NCAPS], bf16, tag="cwu"
                    )
                    nc.scalar.activation(
                        out=cwu[:], in_=dist[:], func=AF.Abs_reciprocal_sqrt
                    )
                    cwu_tiles.append(cwu)

                # smalls pass 3: alpha = cwu / sum_t cwu (doubled unless last)
                for g in range(NGRP):
                    cwu = cwu_tiles[g]
                    cwsum = small_pool.tile([128, GRP, NCAPS], f32, tag="cwsum")
                    nc.vector.tensor_reduce(
                        out=cwsum[:],
                        in_=cwu[:].transpose([0, 1, 3, 2]),
                        axis=X,
                        op=ADD,
                    )
                    rs = small_pool.tile([128, GRP, NCAPS], f32, tag="rs")
                    nc.vector.reciprocal_approx_fast(rs[:], cwsum[:])
                    rsb = small_pool.tile([128, GRP, NCAPS], bf16, tag="rsb")
                    nc.vector.tensor_scalar_mul(
                        rsb[:], rs[:], 1.0 if last else 2.0
                    )
                    nc.vector.tensor_mul(
                        alpha[g][:],
                        cwu[:],
                        rsb[:].unsqueeze(2).broadcast_to((128, GRP, KK, NCAPS)),
                    )

                # phase C/D: o' = sum_t alpha_t p_t
                if not last:
                    for g in range(NGRP):
                        wprod = big_pool.tile(
                            [128, GRP, KK, CCH, NCAPS], bf16, tag="big"
                        )
                        ab = alpha[g][:].unsqueeze(3).broadcast_to(
                            (128, GRP, KK, CCH, NCAPS)
                        )
                        nc.vector.tensor_mul(wprod[:], priors[g][:], ab)
                        wp = wprod[:].rearrange("p c t a b -> p c t (a b)")
                        wh = wh_pool.tile([128, GRP, 4, NC], bf16, tag="wh")
                        nc.vector.tensor_add(wh[:], wp[:, :, 0:4], wp[:, :, 4:8])
                        nc.vector.tensor_add(
                            wh[:, :, 0:2], wh[:, :, 0:2], wh[:, :, 2:4]
                        )
                        nc.vector.tensor_add(wh[:, :, 0], wh[:, :, 0], wh[:, :, 1])
                        nc.vector.tensor_add(o2[g][:], wh[:, :, 0], wp[:, :, 8])
                else:
                    # last iteration: per sub-chunk; output transposes overlap
                    # the other sub-chunks' weighted sums
                    for g in range(NGRP):
                        for cc in range(GRP):
                            ch = GRP * g + cc
                            wprod = big_pool.tile(
                                [128, KK, CCH, NCAPS], bf16, tag="big"
                            )
                            ab = alpha[g][:, cc].unsqueeze(2).broadcast_to(
                                (128, KK, CCH, NCAPS)
                            )
                            nc.vector.tensor_mul(wprod[:], priors[g][:, cc], ab)
                            wp = wprod[:].rearrange("p t a b -> p t (a b)")
                            wh = wh_pool.tile([128, 4, NC], bf16, tag="wh")
                            nc.vector.tensor_add(wh[:], wp[:, 0:4], wp[:, 4:8])
                            nc.vector.tensor_add(
                                wh[:, 0:2], wh[:, 0:2], wh[:, 2:4]
                            )
                            nc.vector.tensor_add(wh[:, 0], wh[:, 0], wh[:, 1])
                            onat = ost_pool.tile([128, NC], f32, tag="onat")
                            nc.vector.tensor_add(
                                onat[:].rearrange(
                                    "p (cap cch) -> p cch cap", cch=CCH
                                ),
                                wh[:, 0].rearrange(
                                    "p (cch cap) -> p cch cap", cch=CCH
                                ),
                                wp[:, 8].rearrange(
                                    "p (cch cap) -> p cch cap", cch=CCH
                                ),
                            )
                            ot = ost_pool.tile([128, 4, 128], f32, tag="ostage")
                            for blk in range(4):
                                tp = tpp.tile([128, 128], f32)
                                nc.tensor.transpose(
                                    tp[:],
                                    onat[:, 128 * blk : 128 * (blk + 1)],
                                    ident[:],
                                )
                                nc.scalar.copy(out=ot[:, blk], in_=tp[:])
                            nc.sync.dma_start(
                                out=out_d[
                                    :, 128 * ch : 128 * (ch + 1)
                                ].rearrange("(blk n) q -> n blk q", blk=4),
                                in_=ot[:],
                            )
    nc.compile()
    return nc


def _get_nc(with_bias: bool):
    key = ("nc", with_bias)
    if key not in _cache:
        _cache[key] = _build(with_bias)
    return _cache[key]


def kernel(input, weight, bias, _trace=False):
    from concourse.bass_utils import run_bass_kernel_spmd

    input = np.ascontiguousarray(np.asarray(input, dtype=np.float32))
    w = np.ascontiguousarray(
        np.asarray(weight, dtype=np.float32).reshape(KK, C, NC)
    )
    b = np.ascontiguousarray(np.asarray(bias, dtype=np.float32).reshape(KK, NC))
    with_bias = bool(np.any(b))

    nc = _get_nc(with_bias)
    in_maps = [
        {"x": np.ascontiguousarray(input[i]), "w": w, "b": b} for i in range(B)
    ]
    res = run_bass_kernel_spmd(
        nc, in_maps, core_ids=list(range(B)), trace=_trace
    )
    _cache["last_result"] = res
    out = np.stack(
        [r["out"].reshape(NC, H, W) for r in res.results], axis=0
    )
    return out


# revision 13
# speedup vs baseline: 1.2485x; 1.0369x over previous
"""Trainium2 Bass kernel for nn_CapsuleLayer (capsule conv + 3-iter routing).

Reference (per batch image, C=128, H=W=32, K=3, pad=1):
  priors[h,w,t,nc] = sum_c x_pad[c,h+i,w+j] * W[t,c,nc] + b[t,nc]
  o = mean_t priors
  3x: d2 = sum_cch (o - p_t)^2 ; cw = rsqrt(d2 + 1e-4)
      cw = cw / sum_t cw ; o = sum_t cw_t p_t
  out[nc,h,w] = o

Sharding: data-parallel over batch; 8 cores, one image each; weight/bias
replicated; no collectives.

v3 structure (vs group-serial baseline):
- All 4 position-groups advance in LOCKSTEP PHASES per routing iteration
  (A/B: products + cch-reduction to s; smalls: e2/dist/rsqrt/alpha;
  C/D: weighted sum).  The ACT hops (e2b, rsqrt) and PSUM round-trips of
  group g hide behind DVE work on groups g+1..g+3 instead of stalling a
  serial per-group chain.
- All bulk elementwise work stays on DVE at 2x_1P bf16 (GPSIMD tensor ops
  measured to *steal the shared SBUF port* and slow DVE by ~60% -- not
  used).  PSUM drains + square + rsqrt on ACT as in the baseline.
- bf16 on-chip, fp32 PSUM matmul accumulation; rel err ~8e-3 < 2e-2 gate.
"""

import numpy as np

C = 128
H = W = 32
B = 8
KK = 9
NCAPS = 32
CCH = 16
NC = NCAPS * CCH  # 512
NIT = 3
NPOS = H * W
CHUNK = 128
GRP = 2  # position-chunks per group
NGRP = NPOS // (CHUNK * GRP)  # 4 groups
PADW = 34

_cache = {}


def _build(with_bias: bool):
    import concourse.bass as bass
    import concourse.tile as tile
    from concourse import bacc, mybir
    from concourse.masks import make_identity

    f32 = mybir.dt.float32
    bf16 = mybir.dt.bfloat16
    X = mybir.AxisListType.X
    ADD = mybir.AluOpType.add
    AF = mybir.ActivationFunctionType

    nc = bacc.Bacc()
    x_d = nc.dram_tensor("x", [C, H, W], f32, kind="ExternalInput")
    w_d = nc.dram_tensor("w", [KK, C, NC], f32, kind="ExternalInput")
    b_d = nc.dram_tensor("b", [KK, NC], f32, kind="ExternalInput")
    out_d = nc.dram_tensor("out", [NC, NPOS], f32, kind="ExternalOutput")

    with tile.TileContext(nc) as tc:
        with (
            tc.tile_pool(name="singles", bufs=1) as singles,
            tc.tile_pool(name="wload", bufs=3) as wload,
            tc.tile_pool(name="big", bufs=2) as big_pool,
            tc.tile_pool(name="h1p", bufs=2) as h1_pool,
            tc.tile_pool(name="wh", bufs=2) as wh_pool,
            tc.tile_pool(name="small", bufs=2) as small_pool,
            tc.tile_pool(name="gstate", bufs=1) as gstate,
            tc.tile_pool(name="ost", bufs=2) as ost_pool,
            tc.tile_pool(name="pp", bufs=4, space="PSUM") as pp,
            tc.tile_pool(name="mp", bufs=2, space="PSUM") as mp,
            tc.tile_pool(name="tpp", bufs=2, space="PSUM") as tpp,
        ):
            # ---- stage inputs directly from HBM (gpsimd cast-DMAs) ----
            # xs[j][c, r*32+w] = x_pad[c, r, w+j] = x[c, r-1, w+j-1] in-range.
            # W loads go first on the gpsimd queue so tap-0 matmuls can start
            # as soon as the xs top halves land; each xs is row-split so
            # chunks 0-3 only wait for the top half.
            wraws = []
            for t in range(KK):
                wt = wload.tile([C, NC], bf16, tag="wraw", name=f"wr{t}")
                nc.gpsimd.dma_start(out=wt[:], in_=w_d[t])
                wraws.append(wt)

            xs = []
            xcol = [(1, 32, 0, 31), (0, 32, 0, 32), (0, 31, 1, 32)]
            for j in range(3):
                xj = singles.tile([C, PADW * W], bf16, tag=f"xs{j}")
                xjv = xj[:].rearrange("p (r w) -> p r w", r=PADW)
                nc.gpsimd.memset(xjv[:, 0], 0.0)
                nc.gpsimd.memset(xjv[:, PADW - 1], 0.0)
                d0, d1, s0, s1 = xcol[j]
                if j == 0:
                    nc.gpsimd.memset(xjv[:, 1 : PADW - 1, 0], 0.0)
                if j == 2:
                    nc.gpsimd.memset(xjv[:, 1 : PADW - 1, W - 1], 0.0)
                nc.gpsimd.dma_start(
                    out=xjv[:, 1:18, d0:d1], in_=x_d[:, 0:17, s0:s1]
                )
                xs.append(xj)
            for j in range(3):
                d0, d1, s0, s1 = xcol[j]
                xjv = xs[j][:].rearrange("p (r w) -> p r w", r=PADW)
                nc.gpsimd.dma_start(
                    out=xjv[:, 18 : H + 1, d0:d1], in_=x_d[:, 17:H, s0:s1]
                )

            # per-tap permute (cap,cch)->(cch,cap) on the idle DVE head so
            # the matmul rhs is contiguous
            wsb = []
            for t in range(KK):
                wp_t = singles.tile([C, CCH, NCAPS], bf16, tag=f"wsbp{t}")
                nc.vector.tensor_copy(
                    out=wp_t[:],
                    in_=wraws[t][:].rearrange(
                        "p (cap cch) -> p cch cap", cch=CCH
                    ),
                )
                wsb.append(wp_t)

            ident = singles.tile([128, 128], f32)
            make_identity(nc, ident[:])

            eps = singles.tile([128, 1], f32)
            nc.gpsimd.memset(eps, 1e-4)

            if with_bias:
                braw = singles.tile([1, KK, NC], bf16)
                nc.gpsimd.dma_start(out=braw[:], in_=b_d[:].unsqueeze(0))
                bsb = singles.tile([1, KK, CCH, NCAPS], bf16)
                nc.scalar.copy(
                    out=bsb[:],
                    in_=braw[:].rearrange("p t (cap cch) -> p t cch cap", cch=CCH),
                )
                ones = singles.tile([1, CHUNK], bf16)
                nc.gpsimd.memset(ones, 1.0)

            # persistent per-group state
            priors = [
                singles.tile(
                    [128, GRP, KK, CCH, NCAPS], bf16,
                    tag=f"pr{g}", name=f"pr{g}",
                )
                for g in range(NGRP)
            ]
            ntile = [
                singles.tile(
                    [128, GRP, KK, NCAPS], bf16, tag=f"nt{g}", name=f"nt{g}"
                )
                for g in range(NGRP)
            ]
            o2 = [
                gstate.tile([128, GRP, NC], bf16, tag=f"o2{g}", name=f"o2{g}")
                for g in range(NGRP)
            ]
            alpha = [
                gstate.tile(
                    [128, GRP, KK, NCAPS], bf16, tag=f"al{g}", name=f"al{g}"
                )
                for g in range(NGRP)
            ]

            # ---- priors + mean per chunk (PE + ACT drains, pipelined) ----
            def emit_chunk(ch):
                g, cc = divmod(ch, GRP)
                om = mp.tile([128, NC], f32)
                for t in range(KK):
                    i, j = divmod(t, 3)
                    ps = pp.tile([128, NC], f32)
                    lhsT = xs[j][:, 128 * ch + 32 * i : 128 * ch + 32 * i + 128]
                    rhs = wsb[t][:].rearrange("p a b -> p (a b)")
                    if with_bias:
                        nc.tensor.matmul(ps[:], lhsT, rhs, start=True, stop=False)
                        brhs = bsb[:, t].rearrange("p a b -> p (a b)")
                        nc.tensor.matmul(
                            ps[:], ones[:], brhs, start=False, stop=True
                        )
                    else:
                        nc.tensor.matmul(ps[:], lhsT, rhs, start=True, stop=True)
                    nc.tensor.matmul(
                        om[:], lhsT, rhs, start=(t == 0), stop=(t == KK - 1)
                    )
                    if with_bias:
                        nc.tensor.matmul(
                            om[:], ones[:], brhs, start=False, stop=False,
                            skip_group_check=True,
                        )
                    psv = ps[:].rearrange("p (a b) -> p a b", a=CCH)
                    if ch == 0:
                        # DVE is idle before routing starts: help drain the
                        # first chunk's PSUM so group 0 can begin sooner
                        nc.vector.tensor_copy(out=priors[g][:, cc, t], in_=psv)
                    else:
                        nc.scalar.copy(out=priors[g][:, cc, t], in_=psv)
                # o2 = 2*mean = (2/9) sum_t priors  (bf16); on DVE so the
                # A-product never waits behind ACT's drain queue
                nc.vector.tensor_scalar_mul(o2[g][:, cc], om[:], 2.0 / KK)

            # ---- routing: 3 iterations, phase-interleaved across groups;
            # iteration 0's A/B rolls together with the priors pipeline so
            # ACT drains/squares for group g+1 overlap group g's DVE work
            def emit_ab(g, it):
                    tprod = big_pool.tile(
                        [128, GRP, KK, CCH, NCAPS], bf16, tag="big"
                    )
                    ob = (
                        o2[g][:]
                        .rearrange("p c (a b) -> p c a b", a=CCH)
                        .unsqueeze(2)
                        .broadcast_to((128, GRP, KK, CCH, NCAPS))
                    )
                    nc.vector.tensor_mul(tprod[:], priors[g][:], ob)
                    h1 = h1_pool.tile([128, GRP, KK, 8, NCAPS], bf16, tag="h1")
                    nc.vector.tensor_add(
                        h1[:], tprod[:, :, :, 0:8], tprod[:, :, :, 8:16]
                    )
                    nc.vector.tensor_add(
                        h1[:, :, :, 0:4], h1[:, :, :, 0:4], h1[:, :, :, 4:8]
                    )
                    nc.vector.tensor_add(
                        h1[:, :, :, 0:2], h1[:, :, :, 0:2], h1[:, :, :, 2:4]
                    )
                    s = small_pool.tile(
                        [128, GRP, KK, NCAPS], bf16, tag=f"s{g}", name=f"s{g}"
                    )
                    nc.vector.tensor_add(s[:], h1[:, :, :, 0], h1[:, :, :, 1])

                    if it == 0:
                        # ntile = sum_cch p^2: square on ACT (overlaps the
                        # next group's DVE product), halvings on DVE
                        tsq = big_pool.tile(
                            [128, GRP, KK, CCH, NCAPS], bf16, tag="big"
                        )
                        nc.scalar.activation(
                            out=tsq[:], in_=priors[g][:], func=AF.Square
                        )
                        nh = h1_pool.tile(
                            [128, GRP, KK, 8, NCAPS], bf16, tag="h1"
                        )
                        nc.vector.tensor_add(
                            nh[:], tsq[:, :, :, 0:8], tsq[:, :, :, 8:16]
                        )
                        nc.vector.tensor_add(
                            nh[:, :, :, 0:4], nh[:, :, :, 0:4], nh[:, :, :, 4:8]
                        )
                        nc.vector.tensor_add(
                            nh[:, :, :, 0:2], nh[:, :, :, 0:2], nh[:, :, :, 2:4]
                        )
                        nc.vector.tensor_add(
                            ntile[g][:], nh[:, :, :, 0], nh[:, :, :, 1]
                        )
                    return s

            for it in range(NIT):
                last = it == NIT - 1
                s_tiles = []
                if it == 0:
                    for g in range(NGRP):
                        emit_chunk(GRP * g)
                        emit_chunk(GRP * g + 1)
                        s_tiles.append(emit_ab(g, it))
                else:
                    for g in range(NGRP):
                        s_tiles.append(emit_ab(g, it))

                # smalls pass 1: e2 = sum_t alpha_t s_t ; e2b = scale*e2+eps
                e2b_tiles = []
                for g in range(NGRP):
                    s = s_tiles[g]
                    e2 = small_pool.tile([128, GRP, NCAPS], f32, tag="e2")
                    if it == 0:
                        red = s
                        escale = 1.0 / (2 * KK)
                    else:
                        tm = small_pool.tile(
                            [128, GRP, KK, NCAPS], bf16, tag="tm"
                        )
                        nc.vector.tensor_mul(tm[:], alpha[g][:], s[:])
                        red = tm
                        escale = 0.25
                    eh = small_pool.tile([128, GRP, 4, NCAPS], bf16, tag="eh")
                    nc.vector.tensor_add(eh[:], red[:, :, 0:4], red[:, :, 4:8])
                    nc.vector.tensor_add(
                        eh[:, :, 0:2], eh[:, :, 0:2], eh[:, :, 2:4]
                    )
                    nc.vector.tensor_add(eh[:, :, 0], eh[:, :, 0], eh[:, :, 1])
                    nc.vector.tensor_add(e2[:], eh[:, :, 0], red[:, :, 8])
                    e2b = small_pool.tile([128, GRP, NCAPS], bf16, tag="e2b")
                    nc.scalar.activation(
                        out=e2b[:], in_=e2[:], func=AF.Identity,
                        bias=eps[:], scale=escale,
                    )
                    e2b_tiles.append(e2b)

                # smalls pass 2: dist = (n - s) + e2b ; cwu = rsqrt(dist)
                cwu_tiles = []
                for g in range(NGRP):
                    dist = small_pool.tile(
                        [128, GRP, KK, NCAPS], bf16, tag="dist"
                    )
                    nc.vector.tensor_sub(dist[:], ntile[g][:], s_tiles[g][:])
                    nc.vector.tensor_add(
                        dist[:],
                        dist[:],
                        e2b_tiles[g][:]
                        .unsqueeze(2)
                        .broadcast_to((128, GRP, KK, NCAPS)),
                    )
                    cwu = small_pool.tile(
                        [128, GRP, KK, NCAPS], bf16, tag="cwu"
                    )
                    nc.scalar.activation(
                        out=cwu[:], in_=dist[:], func=AF.Abs_reciprocal_sqrt
                    )
                    cwu_tiles.append(cwu)

                # smalls pass 3: alpha = cwu / sum_t cwu (doubled unless last)
                for g in range(NGRP):
                    cwu = cwu_tiles[g]
                    cwsum = small_pool.tile([128, GRP, NCAPS], f32, tag="cwsum")
                    ch_ = small_pool.tile([128, GRP, 4, NCAPS], bf16, tag="eh")
                    nc.vector.tensor_add(ch_[:], cwu[:, :, 0:4], cwu[:, :, 4:8])
                    nc.vector.tensor_add(
                        ch_[:, :, 0:2], ch_[:, :, 0:2], ch_[:, :, 2:4]
                    )
                    nc.vector.tensor_add(ch_[:, :, 0], ch_[:, :, 0], ch_[:, :, 1])
                    nc.vector.tensor_add(cwsum[:], ch_[:, :, 0], cwu[:, :, 8])
                    rs = small_pool.tile([128, GRP, NCAPS], f32, tag="rs")
                    nc.vector.reciprocal_approx_fast(rs[:], cwsum[:])
                    rsb = small_pool.tile([128, GRP, NCAPS], bf16, tag="rsb")
                    nc.vector.tensor_scalar_mul(
                        rsb[:], rs[:], 1.0 if last else 2.0
                    )
                    nc.vector.tensor_mul(
                        alpha[g][:],
                        cwu[:],
                        rsb[:].unsqueeze(2).broadcast_to((128, GRP, KK, NCAPS)),
                    )

                # phase C/D: o' = sum_t alpha_t p_t
                if not last:
                    for g in range(NGRP):
                        wprod = big_pool.tile(
                            [128, GRP, KK, CCH, NCAPS], bf16, tag="big"
                        )
                        ab = alpha[g][:].unsqueeze(3).broadcast_to(
                            (128, GRP, KK, CCH, NCAPS)
                        )
                        nc.vector.tensor_mul(wprod[:], priors[g][:], ab)
                        wp = wprod[:].rearrange("p c t a b -> p c t (a b)")
                        wh = wh_pool.tile([128, GRP, 4, NC], bf16, tag="wh")
                        nc.vector.tensor_add(wh[:], wp[:, :, 0:4], wp[:, :, 4:8])
                        nc.vector.tensor_add(
                            wh[:, :, 0:2], wh[:, :, 0:2], wh[:, :, 2:4]
                        )
                        nc.vector.tensor_add(wh[:, :, 0], wh[:, :, 0], wh[:, :, 1])
                        nc.vector.tensor_add(o2[g][:], wh[:, :, 0], wp[:, :, 8])
                else:
                    # last iteration: per sub-chunk; output transposes overlap
                    # the other sub-chunks' weighted sums
                    for g in range(NGRP):
                        for cc in range(GRP):
                            ch = GRP * g + cc
                            wprod = big_pool.tile(
                                [128, KK, CCH, NCAPS], bf16, tag="big"
                            )
                            ab = alpha[g][:, cc].unsqueeze(2).broadcast_to(
                                (128, KK, CCH, NCAPS)
                            )
                            nc.vector.tensor_mul(wprod[:], priors[g][:, cc], ab)
                            wp = wprod[:].rearrange("p t a b -> p t (a b)")
                            wh = wh_pool.tile([128, 4, NC], bf16, tag="wh")
                            nc.vector.tensor_add(wh[:], wp[:, 0:4], wp[:, 4:8])
                            nc.vector.tensor_add(
                                wh[:, 0:2], wh[:, 0:2], wh[:, 2:4]
                            )
                            nc.vector.tensor_add(wh[:, 0], wh[:, 0], wh[:, 1])
                            onat = ost_pool.tile([128, NC], f32, tag="onat")
                            nc.vector.tensor_add(
                                onat[:].rearrange(
                                    "p (cap cch) -> p cch cap", cch=CCH
                                ),
                                wh[:, 0].rearrange(
                                    "p (cch cap) -> p cch cap", cch=CCH
                                ),
                                wp[:, 8].rearrange(
                                    "p (cch cap) -> p cch cap", cch=CCH
                                ),
                            )
                            ot = ost_pool.tile([128, 4, 128], f32, tag="ostage")
                            for blk in range(4):
                                tp = tpp.tile([128, 128], f32)
                                nc.tensor.transpose(
                                    tp[:],
                                    onat[:, 128 * blk : 128 * (blk + 1)],
                                    ident[:],
                                )
                                nc.scalar.copy(out=ot[:, blk], in_=tp[:])
                            nc.sync.dma_start(
                                out=out_d[
                                    :, 128 * ch : 128 * (ch + 1)
                                ].rearrange("(blk n) q -> n blk q", blk=4),
                                in_=ot[:],
                            )
    nc.compile()
    return nc


def _get_nc(with_bias: bool):
    key = ("nc", with_bias)
    if key not in _cache:
        _cache[key] = _build(with_bias)
    return _cache[key]


def kernel(input, weight, bias, _trace=False):
    from concourse.bass_utils import run_bass_kernel_spmd

    input = np.ascontiguousarray(np.asarray(input, dtype=np.float32))
    w = np.ascontiguousarray(
        np.asarray(weight, dtype=np.float32).reshape(KK, C, NC)
    )
    b = np.ascontiguousarray(np.asarray(bias, dtype=np.float32).reshape(KK, NC))
    with_bias = bool(np.any(b))

    nc = _get_nc(with_bias)
    in_maps = [
        {"x": np.ascontiguousarray(input[i]), "w": w, "b": b} for i in range(B)
    ]
    res = run_bass_kernel_spmd(
        nc, in_maps, core_ids=list(range(B)), trace=_trace
    )
    _cache["last_result"] = res
    out = np.stack(
        [r["out"].reshape(NC, H, W) for r in res.results], axis=0
    )
    return out
